# revision 11
# baseline (speedup 1.0000x reference)
"""Trainium2 Bass kernel for causal MHA (B=4, L=2048, D=1024, H=16), 8 cores.

Sharding: data-parallel over batch (4) x tensor-parallel over heads (2).
Each core handles one batch element and 8 heads.

v2 design (vs the f32r baseline):
  - all matmul operands in bf16 (fp32 PSUM accumulate): halves DMA bytes,
    SBUF footprint and LDWEIGHTS time (FWL), doubles DVE throughput
  - V stays resident in SBUF (no DRAM bounce)
  - x loaded once
  - exp batched: one ACT call per key-block covering BOTH heads' score
    tiles ([P, 2, 512] PSUM span -> FD=1024)
  - software-pipelined emission: V-proj/QK-proj chunks for the next pair
    are interleaved into the current pair's attention so the PE stream
    stays dense (HAM stays warm)
"""

import numpy as np

import concourse.bass as bass
import concourse.bacc as bacc
import concourse.mybir as mybir
import concourse.tile as tile

P = 128
HD = 64  # head dim

F32 = mybir.dt.float32
F32R = mybir.dt.float32r
BF16 = mybir.dt.bfloat16


def _chunks(start, end, bank=512):
    out = []
    c = start
    while c < end:
        n = min(bank - (c % bank), end - c)
        out.append((c, n))
        c += n
    return out


def build_mha_nc(L, D, HEADS):
    """Build the per-core Bass program. One batch element, HEADS heads."""
    DBLK = D // P          # contraction blocks for projections
    KB = L // P            # key blocks
    MC = L // 512          # token chunks for projections
    EQK = 2 * HEADS * HD   # q+k output channels per core
    ET = EQK // P          # qk e-tiles (per pair: q-block, k-block)
    EV = HEADS * HD        # v output channels per core
    PAIRS = HEADS // 2
    QS = 512               # q-span per AV-psum accumulation
    NQ = L // QS
    RPH = QS // P
    assert L % 512 == 0 and D % P == 0 and HEADS % 2 == 0 and NQ == MC

    nc = bacc.Bacc("TRN2", target_bir_lowering=False, debug=False,
                   enable_asserts=False)

    xT = nc.dram_tensor("xT", [D, L], BF16, kind="ExternalInput").ap()
    wT = nc.dram_tensor("wT", [D, EQK + EV], BF16, kind="ExternalInput").ap()
    bqk = nc.dram_tensor("bqk", [P, ET], F32, kind="ExternalInput").ap()
    vb = nc.dram_tensor("vb", [P, EV], F32, kind="ExternalInput").ap()
    woT = nc.dram_tensor("woT", [EV, D], BF16, kind="ExternalInput").ap()
    ob = nc.dram_tensor("ob", [P, D], F32, kind="ExternalInput").ap()
    tri = nc.dram_tensor("tri", [P, P], BF16, kind="ExternalInput").ap()
    onep = nc.dram_tensor("onep", [P, HD], F32R, kind="ExternalInput").ap()
    out = nc.dram_tensor("out", [L, D], F32, kind="ExternalOutput").ap()

    scale = 1.0 / float(np.sqrt(HD))

    with tile.TileContext(nc) as tc:
        import contextlib
        ctx = contextlib.ExitStack()
        with ctx:
            consts = ctx.enter_context(tc.tile_pool(name="consts", bufs=1))
            w_pool = ctx.enter_context(tc.tile_pool(name="w", bufs=1))
            x_pool = ctx.enter_context(tc.tile_pool(name="x", bufs=1))
            qk_pool = ctx.enter_context(tc.tile_pool(name="qk", bufs=1))
            vh_pool = ctx.enter_context(tc.tile_pool(name="vh", bufs=1))
            ex_pool = ctx.enter_context(tc.tile_pool(name="ex", bufs=6))
            attn_pool = ctx.enter_context(tc.tile_pool(name="attn", bufs=1))
            outst_pool = ctx.enter_context(tc.tile_pool(name="outst", bufs=4))
            den_pool = ctx.enter_context(tc.tile_pool(name="den", bufs=4))
            recl_pool = ctx.enter_context(tc.tile_pool(name="recl", bufs=2))
            drow_pool = ctx.enter_context(tc.tile_pool(name="drow", bufs=2))
            tmp_pool = ctx.enter_context(tc.tile_pool(name="tmp", bufs=4))
            st_ps = ctx.enter_context(
                tc.tile_pool(name="st_ps", bufs=2, space="PSUM"))
            av_ps = ctx.enter_context(
                tc.tile_pool(name="av_ps", bufs=2, space="PSUM"))
            mm_ps = ctx.enter_context(
                tc.tile_pool(name="mm_ps", bufs=2, space="PSUM"))

            # ---- constants / weights (small, gpsimd queue) ----
            tri_sb = consts.tile([P, P], BF16, name="tri_sb")
            nc.gpsimd.dma_start(out=tri_sb, in_=tri)
            bqk_sb = consts.tile([P, ET], F32, name="bqk_sb")
            nc.gpsimd.dma_start(out=bqk_sb, in_=bqk)
            vb_sb = consts.tile([P, EV], F32, name="vb_sb")
            nc.gpsimd.dma_start(out=vb_sb, in_=vb)
            ob_sb = consts.tile([P, D], F32, name="ob_sb")
            nc.gpsimd.dma_start(out=ob_sb, in_=ob)
            onep_sb = consts.tile([P, HD], F32R, name="onep_sb")
            nc.gpsimd.dma_start(out=onep_sb, in_=onep)

            # ---- x + weights: first compute needs x(mc0) + wv + wqk, so
            # those go first, split across the two DMA queues ----
            xb = x_pool.tile([P, DBLK, L], BF16, name="xb")
            xT_blocked = xT.rearrange("(o p) m -> p o m", p=P)
            wv_sb = w_pool.tile([P, DBLK, EV], BF16, name="wv_sb")
            _wv_src = wT[:, EQK:EQK + EV].rearrange("(o p) e -> p o e", p=P)
            wqk_sb = w_pool.tile([P, DBLK, EQK], BF16, name="wqk_sb")
            _wqk_src = wT[:, 0:EQK].rearrange("(o p) e -> p o e", p=P)

            for _o in range(0, DBLK, 4):
                nc.sync.dma_start(
                    out=xb[:, _o:_o + 4, 0:512],
                    in_=xT_blocked[:, _o:_o + 4, 0:512])
            nc.gpsimd.dma_start(out=wv_sb, in_=_wv_src)
            for _o in range(0, DBLK, 4):
                nc.gpsimd.dma_start(out=wqk_sb[:, _o:_o + 4, :],
                                    in_=_wqk_src[:, _o:_o + 4, :])
            for mc in range(1, MC):
                for _o in range(0, DBLK, 4):
                    nc.sync.dma_start(
                        out=xb[:, _o:_o + 4, mc * 512:(mc + 1) * 512],
                        in_=xT_blocked[:, _o:_o + 4, mc * 512:(mc + 1) * 512])
            wo_sb = w_pool.tile([P, EV // P, D], BF16, name="wo_sb")
            _wo_src = woT.rearrange("(j p) f -> p j f", p=P)
            nc.gpsimd.dma_start(out=wo_sb, in_=_wo_src)

            # ---- persistent SBUF tiles ----
            qk_tiles = [qk_pool.tile([P, L], BF16, name=f"qk_{et}")
                        for et in range(ET)]
            # v resident: [token-in-block, kb, head, chan(+ones)]
            vh = vh_pool.tile([P, KB, HEADS, HD + 1], BF16, name="vh")
            nc.vector.memset(vh[:, :, :, HD:HD + 1], 1.0)
            attn_sb = attn_pool.tile([P, PAIRS, L], BF16, name="attn_sb")

            def v_proj(mc):
                """V projection for the 4 key-blocks of token chunk mc."""
                for mt in range(4):
                    kb = mc * 4 + mt
                    ps = mm_ps.tile([P, 512], F32, name=f"vps_{kb}", tag="mm")
                    for o in range(DBLK):
                        nc.tensor.matmul(
                            ps[:, 0:EV],
                            lhsT=xb[:, o, kb * P:(kb + 1) * P],
                            rhs=wv_sb[:, o, :],
                            start=(o == 0), stop=(o == DBLK - 1))
                    nc.vector.tensor_add(
                        out=vh[:, kb, :, 0:HD],
                        in0=ps[:, 0:EV].rearrange("p (h c) -> p h c", c=HD),
                        in1=vb_sb.rearrange("p (h c) -> p h c", c=HD))

            def qk_proj(pr, mc):
                """QK projection of token chunk mc for pair pr's e-tiles."""
                for et in (2 * pr, 2 * pr + 1):
                    ps = mm_ps.tile([P, 512], F32, name=f"qkps_{et}_{mc}",
                                    tag="mm")
                    for o in range(DBLK):
                        nc.tensor.matmul(
                            ps,
                            lhsT=wqk_sb[:, o, et * P:(et + 1) * P],
                            rhs=xb[:, o, mc * 512:(mc + 1) * 512],
                            start=(o == 0), stop=(o == DBLK - 1))
                    nc.vector.tensor_scalar(
                        out=qk_tiles[et][:, mc * 512:(mc + 1) * 512],
                        in0=ps, scalar1=bqk_sb[:, et:et + 1], scalar2=None,
                        op0=mybir.AluOpType.add)

            # pipeline-order pinning: score MM at st-slot index g must come
            # after the AV MMs at index g-2 (matches the bufs=2 slot FIFO),
            # else the scheduler can greedily invert PE order and deadlock
            st_pipe = {"idx": 0, "avs": {}}

            def attn_q4(pr, q4, avs, tmps):
                """Attention for (pair pr, q-chunk q4): scores+exp+AV."""
                q_tile = qk_tiles[2 * pr]
                k_tile = qk_tiles[2 * pr + 1]
                q0 = q4 * QS
                last_kb = (q0 + QS) // P - 1
                for kb in range(last_kb + 1):
                    s0 = max(0, kb * P - q0)
                    g = st_pipe["idx"]
                    st_pipe["idx"] = g + 1
                    st = st_ps.tile([P, 2, QS], F32,
                                    name=f"st_{pr}_{q4}_{kb}", tag="st")
                    for hh in (0, 1):
                        rows = slice(hh * HD, hh * HD + HD)
                        mm = nc.tensor.matmul(
                            st[:, hh, s0:QS],
                            lhsT=k_tile[rows, kb * P:(kb + 1) * P],
                            rhs=q_tile[rows, q0 + s0:q0 + QS],
                            start=True, stop=True)
                        for av_prev in st_pipe["avs"].get(g - 2, ()):
                            tile.add_dep_helper(
                                mm.ins, av_prev.ins, sync=False,
                                reason="st slot pipeline depth 2")
                    ex = ex_pool.tile([P, 2, QS], BF16,
                                      name=f"ex_{pr}_{q4}_{kb}", tag="ex")
                    nc.scalar.activation(
                        out=ex[:, :, s0:QS], in_=st[:, :, s0:QS],
                        func=mybir.ActivationFunctionType.Exp, scale=scale)
                    if kb * P >= q0:
                        for hh in (0, 1):
                            nc.vector.tensor_mul(
                                out=ex[:, hh, s0:s0 + P],
                                in0=ex[:, hh, s0:s0 + P], in1=tri_sb)
                    av_insts = []
                    for hh in (0, 1):
                        h = 2 * pr + hh
                        av_insts.append(nc.tensor.matmul(
                            avs[hh][:, s0:QS],
                            lhsT=vh[:, kb, h, :],
                            rhs=ex[:, hh, s0:QS],
                            start=(kb == 0), stop=(kb == last_kb)))
                    st_pipe["avs"][g] = av_insts
                    st_pipe["avs"].pop(g - 3, None)
                # evacuate avs: raw copy + denominator row
                for hh in (0, 1):
                    av = avs[hh]
                    if hh == 0:
                        nc.vector.tensor_copy(
                            out=attn_sb[0:HD, pr, q0:q0 + QS],
                            in_=av[0:HD, :])
                        dr = drow_pool.tile([HD + 1, QS], F32R,
                                            name=f"dr_{pr}_{q4}", tag="drow")
                        nc.vector.tensor_copy(out=dr[HD:HD + 1, :],
                                              in_=av[HD:HD + 1, :])
                    else:
                        dr = tmp_pool.tile([HD + 1, QS], F32R,
                                           name=f"tmp_{pr}_{q4}", tag="tmp")
                        nc.vector.tensor_copy(out=dr, in_=av)
                        tmps[q4] = dr
                    nc.sync.dma_start(
                        out=den_tiles[(pr, q4)][hh * RPH:(hh + 1) * RPH, :],
                        in_=dr[HD:HD + 1, :])

            def normalize_q4(pr, q4, tmps):
                """Softmax-normalize (pair pr, q-chunk q4)."""
                den = den_tiles[(pr, q4)]
                recl = recl_pool.tile([HD + 1, QS], F32R,
                                      name=f"recl_{pr}_{q4}", tag="recl")
                with nc.allow_low_precision(
                        reason="fp32r rounding of softmax denom"):
                    nc.vector.reciprocal(out=den, in_=den)
                q0 = q4 * QS
                for hh in (0, 1):
                    base = hh * HD
                    nc.sync.dma_start(
                        out=recl[base:base + 1, :],
                        in_=den[hh * RPH:(hh + 1) * RPH, :])
                    bps = mm_ps.tile([HD, QS], F32,
                                     name=f"bps_{pr}_{hh}_{q4}", tag="mm")
                    nc.tensor.matmul(
                        bps, lhsT=onep_sb[base:base + 1, :],
                        rhs=recl[base:base + 1, :],
                        start=True, stop=True)
                    if hh == 0:
                        sl = attn_sb[0:HD, pr, q0:q0 + QS]
                        nc.vector.tensor_mul(out=sl, in0=sl, in1=bps)
                    else:
                        t = tmps[q4]
                        t2 = tmp_pool.tile([HD, QS], BF16,
                                           name=f"t2_{pr}_{q4}", tag="tmp2")
                        nc.vector.tensor_mul(out=t2, in0=t[0:HD, :], in1=bps)
                        nc.sync.dma_start(
                            out=attn_sb[HD:P, pr, q0:q0 + QS],
                            in_=t2)

            def out_proj(q4):
                """Output projection for the 4 token-blocks of chunk q4."""
                for qt in range(4 * q4, 4 * q4 + 4):
                    for (f0, fn) in _chunks(0, D):
                        ps = mm_ps.tile([P, 512], F32, name=f"ops_{qt}_{f0}",
                                        tag="mm")
                        for j in range(EV // P):
                            nc.tensor.matmul(
                                ps[:, 0:fn],
                                lhsT=attn_sb[:, j, qt * P:(qt + 1) * P],
                                rhs=wo_sb[:, j, f0:f0 + fn],
                                start=(j == 0), stop=(j == EV // P - 1))
                        ot = outst_pool.tile([P, 512], F32,
                                             name=f"ot_{qt}_{f0}", tag="outst")
                        nc.vector.tensor_add(out=ot[:, 0:fn], in0=ps[:, 0:fn],
                                             in1=ob_sb[:, f0:f0 + fn])
                        nc.sync.dma_start(
                            out=out[qt * P:(qt + 1) * P, f0:f0 + fn],
                            in_=ot[:, 0:fn])

            den_tiles = {}

            # ---- pipelined emission ----
            # pair 0: V-proj + own QK-proj feed attention with minimal lag;
            # pair pr>0's QK-proj is emitted as independent PE filler inside
            # pair pr-1's attention steps; out-proj fills pair 3's steps.
            all_tmps = {}
            for pr in range(PAIRS):
                tmps = {}
                all_tmps[pr] = tmps
                for q4 in range(NQ):
                    if pr == 0:
                        v_proj(q4)
                        qk_proj(0, q4)
                    if pr < PAIRS - 1:
                        qk_proj(pr + 1, q4)
                    den_tiles[(pr, q4)] = den_pool.tile(
                        [2 * RPH, P], F32R, name=f"den_{pr}_{q4}", tag="den")
                    avs = [av_ps.tile([HD + 1, QS], F32,
                                      name=f"av_{pr}_{hh}_{q4}", tag="av")
                           for hh in (0, 1)]
                    attn_q4(pr, q4, avs, tmps)
                    normalize_q4(pr, q4, tmps)
                    if pr == PAIRS - 1 and q4 > 0:
                        out_proj(q4 - 1)
            out_proj(NQ - 1)

    nc.compile()
    return nc


def make_core_inputs(x, Wqkv_w, Wqkv_b, out_w, out_b, H, n_tp):
    """Host-side shard + layout prep. Returns list of in_maps (one per core).
    Core c handles batch c // n_tp, head group c % n_tp."""
    import ml_dtypes
    bf16 = ml_dtypes.bfloat16
    B, L, D = x.shape
    hpg = H // n_tp            # heads per core
    PAIRS = hpg // 2
    EQK = 2 * hpg * HD
    EV = hpg * HD
    ET = EQK // P
    tri = np.triu(np.ones((P, P), dtype=np.float32))  # [k, q]: 1 if q >= k
    in_maps = []
    for c in range(B * n_tp):
        b, g = c // n_tp, c % n_tp
        # qk row order: per pair p -> q(2p), q(2p+1), k(2p), k(2p+1)
        qk_rows = []
        for p_ in range(PAIRS):
            for h in (2 * p_, 2 * p_ + 1):
                qk_rows.extend(range(g * hpg * HD + h * HD,
                                     g * hpg * HD + h * HD + HD))
            for h in (2 * p_, 2 * p_ + 1):
                qk_rows.extend(range(D + g * hpg * HD + h * HD,
                                     D + g * hpg * HD + h * HD + HD))
        v_rows = list(range(2 * D + g * hpg * HD, 2 * D + (g + 1) * hpg * HD))
        rows = np.array(qk_rows + v_rows)
        in_maps.append({
            "xT": np.ascontiguousarray(x[b].T).astype(bf16),
            "wT": np.ascontiguousarray(Wqkv_w[rows].T).astype(bf16),
            "bqk": np.ascontiguousarray(
                Wqkv_b[np.array(qk_rows)].reshape(ET, P).T),
            "vb": np.tile(Wqkv_b[np.array(v_rows)], (P, 1)),
            "woT": np.ascontiguousarray(
                out_w[:, g * EV:(g + 1) * EV].T).astype(bf16),
            "ob": (np.tile(out_b, (P, 1)) if g == 0
                   else np.zeros((P, D), np.float32)),
            "tri": tri.astype(bf16),
            "onep": np.ones((P, HD), np.float32),
        })
    return in_maps


_NC_CACHE = {}
LAST_RESULTS = None


def kernel(x, Wqkv_w, Wqkv_b, out_w, out_b):
    global LAST_RESULTS
    x = np.asarray(x, dtype=np.float32)
    Wqkv_w = np.asarray(Wqkv_w, dtype=np.float32)
    Wqkv_b = np.asarray(Wqkv_b, dtype=np.float32)
    out_w = np.asarray(out_w, dtype=np.float32)
    out_b = np.asarray(out_b, dtype=np.float32)

    B, L, D = x.shape
    H = 16
    n_tp = 2
    hpg = H // n_tp

    key = (L, D, hpg)
    if key not in _NC_CACHE:
        _NC_CACHE[key] = build_mha_nc(L, D, hpg)
    nc = _NC_CACHE[key]

    in_maps = make_core_inputs(x, Wqkv_w, Wqkv_b, out_w, out_b, H, n_tp)

    from concourse.bass_utils import run_bass_kernel_spmd
    res = run_bass_kernel_spmd(nc, in_maps, core_ids=list(range(len(in_maps))))
    LAST_RESULTS = res

    out = np.empty((B, L, D), dtype=np.float32)
    for b in range(B):
        out[b] = res.results[n_tp * b]["out"]
        for g in range(1, n_tp):
            out[b] += res.results[n_tp * b + g]["out"]
    return out


if __name__ == "__main__":
    nc = build_mha_nc(2048, 1024, 8)
    print("built OK")


# revision 12
# speedup vs baseline: 1.2664x; 1.2664x over previous
"""Trainium2 Bass kernel for causal MHA (B=4, L=2048, D=1024, H=16), 8 cores.

Sharding: data-parallel over batch (4) x tensor-parallel over heads (2).
Each core handles one batch element and 8 heads.

v2 design (vs the f32r baseline):
  - all matmul operands in bf16 (fp32 PSUM accumulate): halves DMA bytes,
    SBUF footprint and LDWEIGHTS time (FWL), doubles DVE throughput
  - V stays resident in SBUF (no DRAM bounce)
  - x loaded once
  - exp batched: one ACT call per key-block covering BOTH heads' score
    tiles ([P, 2, 512] PSUM span -> FD=1024)
  - software-pipelined emission: V-proj/QK-proj chunks for the next pair
    are interleaved into the current pair's attention so the PE stream
    stays dense (HAM stays warm)
"""

import numpy as np

import concourse.bass as bass
import concourse.bacc as bacc
import concourse.mybir as mybir
import concourse.tile as tile

P = 128
HD = 64  # head dim

F32 = mybir.dt.float32
F32R = mybir.dt.float32r
BF16 = mybir.dt.bfloat16


def _chunks(start, end, bank=512):
    out = []
    c = start
    while c < end:
        n = min(bank - (c % bank), end - c)
        out.append((c, n))
        c += n
    return out


def build_mha_nc(L, D, HEADS):
    """Build the per-core Bass program. One batch element, HEADS heads."""
    DBLK = D // P          # contraction blocks for projections
    KB = L // P            # key blocks
    MC = L // 512          # token chunks for projections
    EQK = 2 * HEADS * HD   # q+k output channels per core
    ET = EQK // P          # qk e-tiles (per pair: q-block, k-block)
    EV = HEADS * HD        # v output channels per core
    PAIRS = HEADS // 2
    QS = 512               # q-span per AV-psum accumulation
    NQ = L // QS
    RPH = QS // P
    assert L % 512 == 0 and D % P == 0 and HEADS % 2 == 0 and NQ == MC

    nc = bacc.Bacc("TRN2", target_bir_lowering=False, debug=False,
                   enable_asserts=False)

    xT = nc.dram_tensor("xT", [D, L], BF16, kind="ExternalInput").ap()
    wT = nc.dram_tensor("wT", [D, EQK + EV], BF16, kind="ExternalInput").ap()
    bqk = nc.dram_tensor("bqk", [P, ET], F32, kind="ExternalInput").ap()
    vb = nc.dram_tensor("vb", [P, EV], F32, kind="ExternalInput").ap()
    woT = nc.dram_tensor("woT", [EV, D], BF16, kind="ExternalInput").ap()
    ob = nc.dram_tensor("ob", [P, D], F32, kind="ExternalInput").ap()
    tri = nc.dram_tensor("tri", [P, P], BF16, kind="ExternalInput").ap()
    onep = nc.dram_tensor("onep", [P, HD], F32R, kind="ExternalInput").ap()
    out = nc.dram_tensor("out", [L, D], F32, kind="ExternalOutput").ap()

    scale = 1.0 / float(np.sqrt(HD))

    with tile.TileContext(nc) as tc:
        import contextlib
        ctx = contextlib.ExitStack()
        with ctx:
            consts = ctx.enter_context(tc.tile_pool(name="consts", bufs=1))
            w_pool = ctx.enter_context(tc.tile_pool(name="w", bufs=1))
            x_pool = ctx.enter_context(tc.tile_pool(name="x", bufs=1))
            qk_pool = ctx.enter_context(tc.tile_pool(name="qk", bufs=1))
            vh_pool = ctx.enter_context(tc.tile_pool(name="vh", bufs=1))
            ex_pool = ctx.enter_context(tc.tile_pool(name="ex", bufs=6))
            attn_pool = ctx.enter_context(tc.tile_pool(name="attn", bufs=1))
            outst_pool = ctx.enter_context(tc.tile_pool(name="outst", bufs=4))
            den_pool = ctx.enter_context(tc.tile_pool(name="den", bufs=4))
            recl_pool = ctx.enter_context(tc.tile_pool(name="recl", bufs=2))
            drow_pool = ctx.enter_context(tc.tile_pool(name="drow", bufs=2))
            tmp_pool = ctx.enter_context(tc.tile_pool(name="tmp", bufs=4))
            st_ps = ctx.enter_context(
                tc.tile_pool(name="st_ps", bufs=2, space="PSUM"))
            av_ps = ctx.enter_context(
                tc.tile_pool(name="av_ps", bufs=2, space="PSUM"))
            mm_ps = ctx.enter_context(
                tc.tile_pool(name="mm_ps", bufs=2, space="PSUM"))

            # ---- constants / weights (small, gpsimd queue) ----
            tri_sb = consts.tile([P, P], BF16, name="tri_sb")
            nc.gpsimd.dma_start(out=tri_sb, in_=tri)
            bqk_sb = consts.tile([P, ET], F32, name="bqk_sb")
            nc.gpsimd.dma_start(out=bqk_sb, in_=bqk)
            vb_sb = consts.tile([P, EV], F32, name="vb_sb")
            nc.gpsimd.dma_start(out=vb_sb, in_=vb)
            ob_sb = consts.tile([P, D], F32, name="ob_sb")
            nc.gpsimd.dma_start(out=ob_sb, in_=ob)
            onep_sb = consts.tile([P, HD], F32R, name="onep_sb")
            nc.gpsimd.dma_start(out=onep_sb, in_=onep)

            # ---- x + weights: first compute needs x(mc0) + wv + wqk, so
            # those go first, split across the two DMA queues ----
            xb = x_pool.tile([P, DBLK, L], BF16, name="xb")
            xT_blocked = xT.rearrange("(o p) m -> p o m", p=P)
            wv_sb = w_pool.tile([P, DBLK, EV], BF16, name="wv_sb")
            _wv_src = wT[:, EQK:EQK + EV].rearrange("(o p) e -> p o e", p=P)
            wqk_sb = w_pool.tile([P, DBLK, EQK], BF16, name="wqk_sb")
            _wqk_src = wT[:, 0:EQK].rearrange("(o p) e -> p o e", p=P)

            for _o in range(0, DBLK, 4):
                nc.sync.dma_start(
                    out=xb[:, _o:_o + 4, 0:512],
                    in_=xT_blocked[:, _o:_o + 4, 0:512])
            nc.gpsimd.dma_start(out=wv_sb, in_=_wv_src)
            for _o in range(0, DBLK, 4):
                nc.gpsimd.dma_start(out=wqk_sb[:, _o:_o + 4, :],
                                    in_=_wqk_src[:, _o:_o + 4, :])
            for mc in range(1, MC):
                for _o in range(0, DBLK, 4):
                    nc.sync.dma_start(
                        out=xb[:, _o:_o + 4, mc * 512:(mc + 1) * 512],
                        in_=xT_blocked[:, _o:_o + 4, mc * 512:(mc + 1) * 512])
            wo_sb = w_pool.tile([P, EV // P, D], BF16, name="wo_sb")
            _wo_src = woT.rearrange("(j p) f -> p j f", p=P)
            nc.gpsimd.dma_start(out=wo_sb, in_=_wo_src)

            # ---- persistent SBUF tiles ----
            qk_tiles = [qk_pool.tile([P, L], BF16, name=f"qk_{et}")
                        for et in range(ET)]
            # v resident: [token-in-block, kb, head, chan(+ones)]
            vh = vh_pool.tile([P, KB, HEADS, HD + 1], BF16, name="vh")
            nc.vector.memset(vh[:, :, :, HD:HD + 1], 1.0)
            attn_sb = attn_pool.tile([P, PAIRS, L], BF16, name="attn_sb")

            def v_proj(mc):
                """V projection for the 4 key-blocks of token chunk mc."""
                for mt in range(4):
                    kb = mc * 4 + mt
                    ps = mm_ps.tile([P, 512], F32, name=f"vps_{kb}", tag="mm")
                    for o in range(DBLK):
                        nc.tensor.matmul(
                            ps[:, 0:EV],
                            lhsT=xb[:, o, kb * P:(kb + 1) * P],
                            rhs=wv_sb[:, o, :],
                            start=(o == 0), stop=(o == DBLK - 1))
                    nc.vector.tensor_add(
                        out=vh[:, kb, :, 0:HD],
                        in0=ps[:, 0:EV].rearrange("p (h c) -> p h c", c=HD),
                        in1=vb_sb.rearrange("p (h c) -> p h c", c=HD))

            def qk_proj(pr, mc):
                """QK projection of token chunk mc for pair pr's e-tiles."""
                for et in (2 * pr, 2 * pr + 1):
                    ps = mm_ps.tile([P, 512], F32, name=f"qkps_{et}_{mc}",
                                    tag="mm")
                    for o in range(DBLK):
                        nc.tensor.matmul(
                            ps,
                            lhsT=wqk_sb[:, o, et * P:(et + 1) * P],
                            rhs=xb[:, o, mc * 512:(mc + 1) * 512],
                            start=(o == 0), stop=(o == DBLK - 1))
                    nc.vector.tensor_scalar(
                        out=qk_tiles[et][:, mc * 512:(mc + 1) * 512],
                        in0=ps, scalar1=bqk_sb[:, et:et + 1], scalar2=None,
                        op0=mybir.AluOpType.add)

            # pipeline-order pinning: score MM at st-slot index g must come
            # after the AV MMs at index g-2 (matches the bufs=2 slot FIFO),
            # else the scheduler can greedily invert PE order and deadlock
            st_pipe = {"idx": 0, "avs": {}}

            def attn_q4(pr, q4, avs, tmps):
                """Attention for (pair pr, q-chunk q4): scores+exp+AV."""
                q_tile = qk_tiles[2 * pr]
                k_tile = qk_tiles[2 * pr + 1]
                q0 = q4 * QS
                last_kb = (q0 + QS) // P - 1
                for kb in range(last_kb + 1):
                    s0 = max(0, kb * P - q0)
                    g = st_pipe["idx"]
                    st_pipe["idx"] = g + 1
                    st = st_ps.tile([P, 2, QS], F32,
                                    name=f"st_{pr}_{q4}_{kb}", tag="st")
                    for hh in (0, 1):
                        rows = slice(hh * HD, hh * HD + HD)
                        mm = nc.tensor.matmul(
                            st[:, hh, s0:QS],
                            lhsT=k_tile[rows, kb * P:(kb + 1) * P],
                            rhs=q_tile[rows, q0 + s0:q0 + QS],
                            start=True, stop=True)
                        for av_prev in st_pipe["avs"].get(g - 2, ()):
                            tile.add_dep_helper(
                                mm.ins, av_prev.ins, sync=False,
                                reason="st slot pipeline depth 2")
                    ex = ex_pool.tile([P, 2, QS], BF16,
                                      name=f"ex_{pr}_{q4}_{kb}", tag="ex")
                    nc.scalar.activation(
                        out=ex[:, :, s0:QS], in_=st[:, :, s0:QS],
                        func=mybir.ActivationFunctionType.Exp, scale=scale)
                    if kb * P >= q0:
                        for hh in (0, 1):
                            nc.vector.tensor_mul(
                                out=ex[:, hh, s0:s0 + P],
                                in0=ex[:, hh, s0:s0 + P], in1=tri_sb)
                    av_insts = []
                    for hh in (0, 1):
                        h = 2 * pr + hh
                        av_insts.append(nc.tensor.matmul(
                            avs[hh][:, s0:QS],
                            lhsT=vh[:, kb, h, :],
                            rhs=ex[:, hh, s0:QS],
                            start=(kb == 0), stop=(kb == last_kb)))
                    st_pipe["avs"][g] = av_insts
                    st_pipe["avs"].pop(g - 3, None)
                # evacuate avs: raw copy + denominator row
                for hh in (0, 1):
                    av = avs[hh]
                    if hh == 0:
                        nc.vector.tensor_copy(
                            out=attn_sb[0:HD, pr, q0:q0 + QS],
                            in_=av[0:HD, :])
                        dr = drow_pool.tile([HD + 1, QS], F32R,
                                            name=f"dr_{pr}_{q4}", tag="drow")
                        nc.vector.tensor_copy(out=dr[HD:HD + 1, :],
                                              in_=av[HD:HD + 1, :])
                    else:
                        dr = tmp_pool.tile([HD + 1, QS], F32R,
                                           name=f"tmp_{pr}_{q4}", tag="tmp")
                        nc.vector.tensor_copy(out=dr, in_=av)
                        tmps[q4] = dr
                    nc.sync.dma_start(
                        out=den_tiles[(pr, q4)][hh * RPH:(hh + 1) * RPH, :],
                        in_=dr[HD:HD + 1, :])

            def normalize_q4(pr, q4, tmps):
                """Softmax-normalize (pair pr, q-chunk q4)."""
                den = den_tiles[(pr, q4)]
                recl = recl_pool.tile([HD + 1, QS], F32R,
                                      name=f"recl_{pr}_{q4}", tag="recl")
                with nc.allow_low_precision(
                        reason="fp32r rounding of softmax denom"):
                    nc.vector.reciprocal(out=den, in_=den)
                q0 = q4 * QS
                for hh in (0, 1):
                    base = hh * HD
                    nc.sync.dma_start(
                        out=recl[base:base + 1, :],
                        in_=den[hh * RPH:(hh + 1) * RPH, :])
                    bps = mm_ps.tile([HD, QS], F32,
                                     name=f"bps_{pr}_{hh}_{q4}", tag="mm")
                    nc.tensor.matmul(
                        bps, lhsT=onep_sb[base:base + 1, :],
                        rhs=recl[base:base + 1, :],
                        start=True, stop=True)
                    if hh == 0:
                        sl = attn_sb[0:HD, pr, q0:q0 + QS]
                        nc.vector.tensor_mul(out=sl, in0=sl, in1=bps)
                    else:
                        t = tmps[q4]
                        t2 = tmp_pool.tile([HD, QS], BF16,
                                           name=f"t2_{pr}_{q4}", tag="tmp2")
                        nc.vector.tensor_mul(out=t2, in0=t[0:HD, :], in1=bps)
                        nc.sync.dma_start(
                            out=attn_sb[HD:P, pr, q0:q0 + QS],
                            in_=t2)

            def out_proj(q4):
                """Output projection for the 4 token-blocks of chunk q4."""
                for qt in range(4 * q4, 4 * q4 + 4):
                    for (f0, fn) in _chunks(0, D):
                        ps = mm_ps.tile([P, 512], F32, name=f"ops_{qt}_{f0}",
                                        tag="mm")
                        for j in range(EV // P):
                            nc.tensor.matmul(
                                ps[:, 0:fn],
                                lhsT=attn_sb[:, j, qt * P:(qt + 1) * P],
                                rhs=wo_sb[:, j, f0:f0 + fn],
                                start=(j == 0), stop=(j == EV // P - 1))
                        ot = outst_pool.tile([P, 512], F32,
                                             name=f"ot_{qt}_{f0}", tag="outst")
                        nc.vector.tensor_add(out=ot[:, 0:fn], in0=ps[:, 0:fn],
                                             in1=ob_sb[:, f0:f0 + fn])
                        nc.sync.dma_start(
                            out=out[qt * P:(qt + 1) * P, f0:f0 + fn],
                            in_=ot[:, 0:fn])

            den_tiles = {}

            # ---- pipelined emission ----
            # pair 0: V-proj + own QK-proj feed attention with minimal lag;
            # pair pr>0's QK-proj is emitted as independent PE filler inside
            # pair pr-1's attention steps; out-proj fills pair 3's steps.
            # normalize is emitted one q4-step late so its bps matmul's
            # mm-pool slot is ready before the FIFO makes later projection
            # matmuls wait on it (the den DMA/reciprocal chain is ~5us)
            all_tmps = {}
            pending_norm = []
            for pr in range(PAIRS):
                tmps = {}
                all_tmps[pr] = tmps
                for q4 in range(NQ):
                    if pr == 0:
                        v_proj(q4)
                        qk_proj(0, q4)
                    if pr < PAIRS - 1:
                        qk_proj(pr + 1, q4)
                    den_tiles[(pr, q4)] = den_pool.tile(
                        [2 * RPH, P], F32R, name=f"den_{pr}_{q4}", tag="den")
                    avs = [av_ps.tile([HD + 1, QS], F32,
                                      name=f"av_{pr}_{hh}_{q4}", tag="av")
                           for hh in (0, 1)]
                    attn_q4(pr, q4, avs, tmps)
                    pending_norm.append((pr, q4))
                    if len(pending_norm) > 1:
                        npr, nq4 = pending_norm.pop(0)
                        normalize_q4(npr, nq4, all_tmps[npr])
                        if npr == PAIRS - 1 and nq4 > 0:
                            out_proj(nq4 - 1)
            for npr, nq4 in pending_norm:
                normalize_q4(npr, nq4, all_tmps[npr])
                if npr == PAIRS - 1 and nq4 > 0:
                    out_proj(nq4 - 1)
            out_proj(NQ - 1)

    nc.compile()
    return nc


def make_core_inputs(x, Wqkv_w, Wqkv_b, out_w, out_b, H, n_tp):
    """Host-side shard + layout prep. Returns list of in_maps (one per core).
    Core c handles batch c // n_tp, head group c % n_tp."""
    import ml_dtypes
    bf16 = ml_dtypes.bfloat16
    B, L, D = x.shape
    hpg = H // n_tp            # heads per core
    PAIRS = hpg // 2
    EQK = 2 * hpg * HD
    EV = hpg * HD
    ET = EQK // P
    tri = np.triu(np.ones((P, P), dtype=np.float32))  # [k, q]: 1 if q >= k
    in_maps = []
    for c in range(B * n_tp):
        b, g = c // n_tp, c % n_tp
        # qk row order: per pair p -> q(2p), q(2p+1), k(2p), k(2p+1)
        qk_rows = []
        for p_ in range(PAIRS):
            for h in (2 * p_, 2 * p_ + 1):
                qk_rows.extend(range(g * hpg * HD + h * HD,
                                     g * hpg * HD + h * HD + HD))
            for h in (2 * p_, 2 * p_ + 1):
                qk_rows.extend(range(D + g * hpg * HD + h * HD,
                                     D + g * hpg * HD + h * HD + HD))
        v_rows = list(range(2 * D + g * hpg * HD, 2 * D + (g + 1) * hpg * HD))
        rows = np.array(qk_rows + v_rows)
        in_maps.append({
            "xT": np.ascontiguousarray(x[b].T).astype(bf16),
            "wT": np.ascontiguousarray(Wqkv_w[rows].T).astype(bf16),
            "bqk": np.ascontiguousarray(
                Wqkv_b[np.array(qk_rows)].reshape(ET, P).T),
            "vb": np.tile(Wqkv_b[np.array(v_rows)], (P, 1)),
            "woT": np.ascontiguousarray(
                out_w[:, g * EV:(g + 1) * EV].T).astype(bf16),
            "ob": (np.tile(out_b, (P, 1)) if g == 0
                   else np.zeros((P, D), np.float32)),
            "tri": tri.astype(bf16),
            "onep": np.ones((P, HD), np.float32),
        })
    return in_maps


_NC_CACHE = {}
LAST_RESULTS = None


def kernel(x, Wqkv_w, Wqkv_b, out_w, out_b):
    global LAST_RESULTS
    x = np.asarray(x, dtype=np.float32)
    Wqkv_w = np.asarray(Wqkv_w, dtype=np.float32)
    Wqkv_b = np.asarray(Wqkv_b, dtype=np.float32)
    out_w = np.asarray(out_w, dtype=np.float32)
    out_b = np.asarray(out_b, dtype=np.float32)

    B, L, D = x.shape
    H = 16
    n_tp = 2
    hpg = H // n_tp

    key = (L, D, hpg)
    if key not in _NC_CACHE:
        _NC_CACHE[key] = build_mha_nc(L, D, hpg)
    nc = _NC_CACHE[key]

    in_maps = make_core_inputs(x, Wqkv_w, Wqkv_b, out_w, out_b, H, n_tp)

    from concourse.bass_utils import run_bass_kernel_spmd
    res = run_bass_kernel_spmd(nc, in_maps, core_ids=list(range(len(in_maps))))
    LAST_RESULTS = res

    out = np.empty((B, L, D), dtype=np.float32)
    for b in range(B):
        out[b] = res.results[n_tp * b]["out"]
        for g in range(1, n_tp):
            out[b] += res.results[n_tp * b + g]["out"]
    return out


if __name__ == "__main__":
    nc = build_mha_nc(2048, 1024, 8)
    print("built OK")


# revision 14
# speedup vs baseline: 1.2778x; 1.0090x over previous
"""Trainium2 Bass kernel for causal MHA (B=4, L=2048, D=1024, H=16), 8 cores.

Sharding: data-parallel over batch (4) x tensor-parallel over heads (2).
Each core handles one batch element and 8 heads.

v2 design (vs the f32r baseline):
  - all matmul operands in bf16 (fp32 PSUM accumulate): halves DMA bytes,
    SBUF footprint and LDWEIGHTS time (FWL), doubles DVE throughput
  - V stays resident in SBUF (no DRAM bounce)
  - x loaded once
  - exp batched: one ACT call per key-block covering BOTH heads' score
    tiles ([P, 2, 512] PSUM span -> FD=1024)
  - software-pipelined emission: V-proj/QK-proj chunks for the next pair
    are interleaved into the current pair's attention so the PE stream
    stays dense (HAM stays warm)
"""

import numpy as np

import concourse.bass as bass
import concourse.bacc as bacc
import concourse.mybir as mybir
import concourse.tile as tile

P = 128
HD = 64  # head dim

F32 = mybir.dt.float32
F32R = mybir.dt.float32r
BF16 = mybir.dt.bfloat16


def _chunks(start, end, bank=512):
    out = []
    c = start
    while c < end:
        n = min(bank - (c % bank), end - c)
        out.append((c, n))
        c += n
    return out


def build_mha_nc(L, D, HEADS):
    """Build the per-core Bass program. One batch element, HEADS heads."""
    DBLK = D // P          # contraction blocks for projections
    KB = L // P            # key blocks
    MC = L // 512          # token chunks for projections
    EQK = 2 * HEADS * HD   # q+k output channels per core
    ET = EQK // P          # qk e-tiles (per pair: q-block, k-block)
    EV = HEADS * HD        # v output channels per core
    PAIRS = HEADS // 2
    QS = 512               # q-span per AV-psum accumulation
    NQ = L // QS
    RPH = QS // P
    assert L % 512 == 0 and D % P == 0 and HEADS % 2 == 0 and NQ == MC

    nc = bacc.Bacc("TRN2", target_bir_lowering=False, debug=False,
                   enable_asserts=False)

    xT = nc.dram_tensor("xT", [D, L], BF16, kind="ExternalInput").ap()
    wT = nc.dram_tensor("wT", [D, EQK + EV], BF16, kind="ExternalInput").ap()
    bqk = nc.dram_tensor("bqk", [P, ET], F32, kind="ExternalInput").ap()
    vb = nc.dram_tensor("vb", [P, EV], F32, kind="ExternalInput").ap()
    woT = nc.dram_tensor("woT", [EV, D], BF16, kind="ExternalInput").ap()
    ob = nc.dram_tensor("ob", [P, D], F32, kind="ExternalInput").ap()
    tri = nc.dram_tensor("tri", [P, P], BF16, kind="ExternalInput").ap()
    onep = nc.dram_tensor("onep", [P, HD], F32R, kind="ExternalInput").ap()
    out = nc.dram_tensor("out", [L, D], F32, kind="ExternalOutput").ap()

    scale = 1.0 / float(np.sqrt(HD))

    with tile.TileContext(nc) as tc:
        import contextlib
        ctx = contextlib.ExitStack()
        with ctx:
            consts = ctx.enter_context(tc.tile_pool(name="consts", bufs=1))
            w_pool = ctx.enter_context(tc.tile_pool(name="w", bufs=1))
            x_pool = ctx.enter_context(tc.tile_pool(name="x", bufs=1))
            qk_pool = ctx.enter_context(tc.tile_pool(name="qk", bufs=1))
            vh_pool = ctx.enter_context(tc.tile_pool(name="vh", bufs=1))
            ex_pool = ctx.enter_context(tc.tile_pool(name="ex", bufs=6))
            attn_pool = ctx.enter_context(tc.tile_pool(name="attn", bufs=1))
            outst_pool = ctx.enter_context(tc.tile_pool(name="outst", bufs=4))
            den_pool = ctx.enter_context(tc.tile_pool(name="den", bufs=4))
            recl_pool = ctx.enter_context(tc.tile_pool(name="recl", bufs=2))
            drow_pool = ctx.enter_context(tc.tile_pool(name="drow", bufs=2))
            tmp_pool = ctx.enter_context(tc.tile_pool(name="tmp", bufs=4))
            st_ps = ctx.enter_context(
                tc.tile_pool(name="st_ps", bufs=2, space="PSUM"))
            av_ps = ctx.enter_context(
                tc.tile_pool(name="av_ps", bufs=2, space="PSUM"))
            mm_ps = ctx.enter_context(
                tc.tile_pool(name="mm_ps", bufs=2, space="PSUM"))

            # ---- x + weights: first compute needs x(mc0) + wv + wqk, so
            # those go first; everything else is ordered by first use.
            # gpsimd queue: weights/consts (+ later the fp32 output);
            # sync queue: x chunks, then the normalize/attn small DMAs.
            xb = x_pool.tile([P, DBLK, L], BF16, name="xb")
            xT_blocked = xT.rearrange("(o p) m -> p o m", p=P)
            wv_sb = w_pool.tile([P, DBLK, EV], BF16, name="wv_sb")
            _wv_src = wT[:, EQK:EQK + EV].rearrange("(o p) e -> p o e", p=P)
            wqk_sb = w_pool.tile([P, DBLK, EQK], BF16, name="wqk_sb")
            _wqk_src = wT[:, 0:EQK].rearrange("(o p) e -> p o e", p=P)

            for _o in range(0, DBLK, 4):
                nc.sync.dma_start(
                    out=xb[:, _o:_o + 4, 0:512],
                    in_=xT_blocked[:, _o:_o + 4, 0:512])
            nc.gpsimd.dma_start(out=wv_sb, in_=_wv_src)
            for _o in range(0, DBLK, 4):
                nc.gpsimd.dma_start(out=wqk_sb[:, _o:_o + 4, :],
                                    in_=_wqk_src[:, _o:_o + 4, :])
            for mc in range(1, MC):
                for _o in range(0, DBLK, 4):
                    nc.sync.dma_start(
                        out=xb[:, _o:_o + 4, mc * 512:(mc + 1) * 512],
                        in_=xT_blocked[:, _o:_o + 4, mc * 512:(mc + 1) * 512])

            vb_sb = consts.tile([P, EV], F32, name="vb_sb")
            nc.gpsimd.dma_start(out=vb_sb, in_=vb)
            bqk_sb = consts.tile([P, ET], F32, name="bqk_sb")
            nc.gpsimd.dma_start(out=bqk_sb, in_=bqk)
            tri_sb = consts.tile([P, P], BF16, name="tri_sb")
            nc.gpsimd.dma_start(out=tri_sb, in_=tri)
            onep_sb = consts.tile([P, HD], F32R, name="onep_sb")
            nc.gpsimd.dma_start(out=onep_sb, in_=onep)
            wo_sb = w_pool.tile([P, EV // P, D], BF16, name="wo_sb")
            _wo_src = woT.rearrange("(j p) f -> p j f", p=P)
            nc.gpsimd.dma_start(out=wo_sb, in_=_wo_src)
            ob_sb = consts.tile([P, D], F32, name="ob_sb")
            nc.gpsimd.dma_start(out=ob_sb, in_=ob)

            # ---- persistent SBUF tiles ----
            qk_tiles = [qk_pool.tile([P, L], BF16, name=f"qk_{et}")
                        for et in range(ET)]
            # v resident: [token-in-block, kb, head, chan(+ones)]
            vh = vh_pool.tile([P, KB, HEADS, HD + 1], BF16, name="vh")
            nc.vector.memset(vh[:, :, :, HD:HD + 1], 1.0)
            attn_sb = attn_pool.tile([P, PAIRS, L], BF16, name="attn_sb")

            def v_proj(mc):
                """V projection for the 4 key-blocks of token chunk mc."""
                for mt in range(4):
                    kb = mc * 4 + mt
                    ps = mm_ps.tile([P, 512], F32, name=f"vps_{kb}", tag="mm")
                    for o in range(DBLK):
                        nc.tensor.matmul(
                            ps[:, 0:EV],
                            lhsT=xb[:, o, kb * P:(kb + 1) * P],
                            rhs=wv_sb[:, o, :],
                            start=(o == 0), stop=(o == DBLK - 1))
                    nc.vector.tensor_add(
                        out=vh[:, kb, :, 0:HD],
                        in0=ps[:, 0:EV].rearrange("p (h c) -> p h c", c=HD),
                        in1=vb_sb.rearrange("p (h c) -> p h c", c=HD))

            def qk_proj(pr, mc):
                """QK projection of token chunk mc for pair pr's e-tiles."""
                for et in (2 * pr, 2 * pr + 1):
                    ps = mm_ps.tile([P, 512], F32, name=f"qkps_{et}_{mc}",
                                    tag="mm")
                    for o in range(DBLK):
                        nc.tensor.matmul(
                            ps,
                            lhsT=wqk_sb[:, o, et * P:(et + 1) * P],
                            rhs=xb[:, o, mc * 512:(mc + 1) * 512],
                            start=(o == 0), stop=(o == DBLK - 1))
                    nc.vector.tensor_scalar(
                        out=qk_tiles[et][:, mc * 512:(mc + 1) * 512],
                        in0=ps, scalar1=bqk_sb[:, et:et + 1], scalar2=None,
                        op0=mybir.AluOpType.add)

            # pipeline-order pinning: score MM at st-slot index g must come
            # after the AV MMs at index g-2 (matches the bufs=2 slot FIFO),
            # else the scheduler can greedily invert PE order and deadlock
            st_pipe = {"idx": 0, "avs": {}}

            def attn_q4(pr, q4, avs, tmps):
                """Attention for (pair pr, q-chunk q4): scores+exp+AV."""
                q_tile = qk_tiles[2 * pr]
                k_tile = qk_tiles[2 * pr + 1]
                q0 = q4 * QS
                last_kb = (q0 + QS) // P - 1
                for kb in range(last_kb + 1):
                    s0 = max(0, kb * P - q0)
                    g = st_pipe["idx"]
                    st_pipe["idx"] = g + 1
                    st = st_ps.tile([P, 2, QS], F32,
                                    name=f"st_{pr}_{q4}_{kb}", tag="st")
                    for hh in (0, 1):
                        rows = slice(hh * HD, hh * HD + HD)
                        mm = nc.tensor.matmul(
                            st[:, hh, s0:QS],
                            lhsT=k_tile[rows, kb * P:(kb + 1) * P],
                            rhs=q_tile[rows, q0 + s0:q0 + QS],
                            start=True, stop=True)
                        for av_prev in st_pipe["avs"].get(g - 2, ()):
                            tile.add_dep_helper(
                                mm.ins, av_prev.ins, sync=False,
                                reason="st slot pipeline depth 2")
                    ex = ex_pool.tile([P, 2, QS], BF16,
                                      name=f"ex_{pr}_{q4}_{kb}", tag="ex")
                    nc.scalar.activation(
                        out=ex[:, :, s0:QS], in_=st[:, :, s0:QS],
                        func=mybir.ActivationFunctionType.Exp, scale=scale)
                    if kb * P >= q0:
                        for hh in (0, 1):
                            nc.vector.tensor_mul(
                                out=ex[:, hh, s0:s0 + P],
                                in0=ex[:, hh, s0:s0 + P], in1=tri_sb)
                    av_insts = []
                    for hh in (0, 1):
                        h = 2 * pr + hh
                        av_insts.append(nc.tensor.matmul(
                            avs[hh][:, s0:QS],
                            lhsT=vh[:, kb, h, :],
                            rhs=ex[:, hh, s0:QS],
                            start=(kb == 0), stop=(kb == last_kb)))
                    st_pipe["avs"][g] = av_insts
                    st_pipe["avs"].pop(g - 3, None)
                # evacuate avs: raw copy + denominator row
                for hh in (0, 1):
                    av = avs[hh]
                    if hh == 0:
                        nc.vector.tensor_copy(
                            out=attn_sb[0:HD, pr, q0:q0 + QS],
                            in_=av[0:HD, :])
                        dr = drow_pool.tile([HD + 1, QS], F32R,
                                            name=f"dr_{pr}_{q4}", tag="drow")
                        nc.vector.tensor_copy(out=dr[HD:HD + 1, :],
                                              in_=av[HD:HD + 1, :])
                    else:
                        dr = tmp_pool.tile([HD + 1, QS], F32R,
                                           name=f"tmp_{pr}_{q4}", tag="tmp")
                        nc.vector.tensor_copy(out=dr, in_=av)
                        tmps[q4] = dr
                    nc.sync.dma_start(
                        out=den_tiles[(pr, q4)][hh * RPH:(hh + 1) * RPH, :],
                        in_=dr[HD:HD + 1, :])

            def normalize_q4(pr, q4, tmps):
                """Softmax-normalize (pair pr, q-chunk q4)."""
                den = den_tiles[(pr, q4)]
                recl = recl_pool.tile([HD + 1, QS], F32R,
                                      name=f"recl_{pr}_{q4}", tag="recl")
                with nc.allow_low_precision(
                        reason="fp32r rounding of softmax denom"):
                    nc.vector.reciprocal(out=den, in_=den)
                q0 = q4 * QS
                for hh in (0, 1):
                    base = hh * HD
                    nc.sync.dma_start(
                        out=recl[base:base + 1, :],
                        in_=den[hh * RPH:(hh + 1) * RPH, :])
                    bps = mm_ps.tile([HD, QS], F32,
                                     name=f"bps_{pr}_{hh}_{q4}", tag="mm")
                    nc.tensor.matmul(
                        bps, lhsT=onep_sb[base:base + 1, :],
                        rhs=recl[base:base + 1, :],
                        start=True, stop=True)
                    if hh == 0:
                        sl = attn_sb[0:HD, pr, q0:q0 + QS]
                        nc.vector.tensor_mul(out=sl, in0=sl, in1=bps)
                    else:
                        t = tmps[q4]
                        t2 = tmp_pool.tile([HD, QS], BF16,
                                           name=f"t2_{pr}_{q4}", tag="tmp2")
                        nc.vector.tensor_mul(out=t2, in0=t[0:HD, :], in1=bps)
                        nc.sync.dma_start(
                            out=attn_sb[HD:P, pr, q0:q0 + QS],
                            in_=t2)

            def out_proj(q4):
                """Output projection for the 4 token-blocks of chunk q4."""
                for qt in range(4 * q4, 4 * q4 + 4):
                    for (f0, fn) in _chunks(0, D):
                        ps = mm_ps.tile([P, 512], F32, name=f"ops_{qt}_{f0}",
                                        tag="mm")
                        for j in range(EV // P):
                            nc.tensor.matmul(
                                ps[:, 0:fn],
                                lhsT=attn_sb[:, j, qt * P:(qt + 1) * P],
                                rhs=wo_sb[:, j, f0:f0 + fn],
                                start=(j == 0), stop=(j == EV // P - 1))
                        ot = outst_pool.tile([P, 512], F32,
                                             name=f"ot_{qt}_{f0}", tag="outst")
                        nc.vector.tensor_add(out=ot[:, 0:fn], in0=ps[:, 0:fn],
                                             in1=ob_sb[:, f0:f0 + fn])
                        nc.gpsimd.dma_start(
                            out=out[qt * P:(qt + 1) * P, f0:f0 + fn],
                            in_=ot[:, 0:fn])

            den_tiles = {}

            # ---- pipelined emission ----
            # pair 0: V-proj + own QK-proj feed attention with minimal lag;
            # pair pr>0's QK-proj is emitted as independent PE filler inside
            # pair pr-1's attention steps; out-proj fills pair 3's steps.
            # normalize is emitted one q4-step late so its bps matmul's
            # mm-pool slot is ready before the FIFO makes later projection
            # matmuls wait on it (the den DMA/reciprocal chain is ~5us)
            all_tmps = {}
            pending_norm = []
            for pr in range(PAIRS):
                tmps = {}
                all_tmps[pr] = tmps
                for q4 in range(NQ):
                    if pr == 0:
                        v_proj(q4)
                        qk_proj(0, q4)
                    if pr < PAIRS - 1:
                        qk_proj(pr + 1, q4)
                    den_tiles[(pr, q4)] = den_pool.tile(
                        [2 * RPH, P], F32R, name=f"den_{pr}_{q4}", tag="den")
                    avs = [av_ps.tile([HD + 1, QS], F32,
                                      name=f"av_{pr}_{hh}_{q4}", tag="av")
                           for hh in (0, 1)]
                    attn_q4(pr, q4, avs, tmps)
                    pending_norm.append((pr, q4))
                    if len(pending_norm) > 1:
                        npr, nq4 = pending_norm.pop(0)
                        normalize_q4(npr, nq4, all_tmps[npr])
                        if npr == PAIRS - 1 and nq4 > 0:
                            out_proj(nq4 - 1)
            for npr, nq4 in pending_norm:
                normalize_q4(npr, nq4, all_tmps[npr])
                if npr == PAIRS - 1 and nq4 > 0:
                    out_proj(nq4 - 1)
            out_proj(NQ - 1)

    nc.compile()
    return nc


def make_core_inputs(x, Wqkv_w, Wqkv_b, out_w, out_b, H, n_tp):
    """Host-side shard + layout prep. Returns list of in_maps (one per core).
    Core c handles batch c // n_tp, head group c % n_tp."""
    import ml_dtypes
    bf16 = ml_dtypes.bfloat16
    B, L, D = x.shape
    hpg = H // n_tp            # heads per core
    PAIRS = hpg // 2
    EQK = 2 * hpg * HD
    EV = hpg * HD
    ET = EQK // P
    tri = np.triu(np.ones((P, P), dtype=np.float32))  # [k, q]: 1 if q >= k
    in_maps = []
    for c in range(B * n_tp):
        b, g = c // n_tp, c % n_tp
        # qk row order: per pair p -> q(2p), q(2p+1), k(2p), k(2p+1)
        qk_rows = []
        for p_ in range(PAIRS):
            for h in (2 * p_, 2 * p_ + 1):
                qk_rows.extend(range(g * hpg * HD + h * HD,
                                     g * hpg * HD + h * HD + HD))
            for h in (2 * p_, 2 * p_ + 1):
                qk_rows.extend(range(D + g * hpg * HD + h * HD,
                                     D + g * hpg * HD + h * HD + HD))
        v_rows = list(range(2 * D + g * hpg * HD, 2 * D + (g + 1) * hpg * HD))
        rows = np.array(qk_rows + v_rows)
        in_maps.append({
            "xT": np.ascontiguousarray(x[b].T).astype(bf16),
            "wT": np.ascontiguousarray(Wqkv_w[rows].T).astype(bf16),
            "bqk": np.ascontiguousarray(
                Wqkv_b[np.array(qk_rows)].reshape(ET, P).T),
            "vb": np.tile(Wqkv_b[np.array(v_rows)], (P, 1)),
            "woT": np.ascontiguousarray(
                out_w[:, g * EV:(g + 1) * EV].T).astype(bf16),
            "ob": (np.tile(out_b, (P, 1)) if g == 0
                   else np.zeros((P, D), np.float32)),
            "tri": tri.astype(bf16),
            "onep": np.ones((P, HD), np.float32),
        })
    return in_maps


_NC_CACHE = {}
LAST_RESULTS = None


def kernel(x, Wqkv_w, Wqkv_b, out_w, out_b):
    global LAST_RESULTS
    x = np.asarray(x, dtype=np.float32)
    Wqkv_w = np.asarray(Wqkv_w, dtype=np.float32)
    Wqkv_b = np.asarray(Wqkv_b, dtype=np.float32)
    out_w = np.asarray(out_w, dtype=np.float32)
    out_b = np.asarray(out_b, dtype=np.float32)

    B, L, D = x.shape
    H = 16
    n_tp = 2
    hpg = H // n_tp

    key = (L, D, hpg)
    if key not in _NC_CACHE:
        _NC_CACHE[key] = build_mha_nc(L, D, hpg)
    nc = _NC_CACHE[key]

    in_maps = make_core_inputs(x, Wqkv_w, Wqkv_b, out_w, out_b, H, n_tp)

    from concourse.bass_utils import run_bass_kernel_spmd
    res = run_bass_kernel_spmd(nc, in_maps, core_ids=list(range(len(in_maps))))
    LAST_RESULTS = res

    out = np.empty((B, L, D), dtype=np.float32)
    for b in range(B):
        out[b] = res.results[n_tp * b]["out"]
        for g in range(1, n_tp):
            out[b] += res.results[n_tp * b + g]["out"]
    return out


if __name__ == "__main__":
    nc = build_mha_nc(2048, 1024, 8)
    print("built OK")
